# revision 66
# baseline (speedup 1.0000x reference)
"""Two-layer GAT (PyG GATConv semantics) as a Bass/Tile kernel on 8 TRN2 NeuronCores.

Strategy (graph/data parallel, dst-sharded):
  - Nodes padded to NPAD=50176, split into 8 contiguous shards of SHARD=6272
    (= 49 blocks x 128). Core k owns dst nodes [k*SHARD, (k+1)*SHARD).
  - Edges (incl. self loops) bucketed by dst shard, sorted by dst, split into
    two streams by src half (gather indices are int16, so gather tables are
    addressed as two halves of < 32768 rows each).
  - P1: every core redundantly computes the full projection table
    tab1[n] = [h(n) bf16(384) | a_src(n) f32(6) | a_dst(n) f32(6) | pad]
    (1024B rows), h = x@W1, a_* = h . att_* via folded weight columns.
    Batched: 8 blocks per xt load, 4 blocks per (strided) tab1 store,
    PSUM->SBUF copies alternate between ACT and DVE.
  - P2: per dst block (128 nodes): dma_gather of tab1 rows for the block's
    in-edges; one-hot matrices U[e,d] built with is_equal compares; UT via
    PE transpose, 4 chunks batched into one PSUM tile + one ACT copy;
    per-edge logits via a tiny PE matmul (UT.T @ a_dst_blk) + gathered
    a_src; fused leaky-relu (scalar_tensor_tensor) + Exp; the ex*h scaling
    done as one broadcast tensor_tensor per gather stream (4D APs);
    aggregation [num | den] = U.T @ [ex*h | ex] accumulated in PSUM over
    the block's chunks; then out = num/den (+bias, ReLU) -> h1. The layer-2
    projection h2e = [h1 @ W2 | a2] is fused into the per-block epilogue.
  - P3: AllGather of h2e into the replicated layer-2 table tab2 (256B rows).
  - P4: same edge machinery for layer 2 (1 head, 32 ch) -> y (own shard).

kernel() takes full inputs, preprocesses indices on the host (sorting /
bucketing / layout only), compiles one SPMD NEFF, runs it on cores 0..7 via
bass_utils.run_bass_kernel_spmd, and concatenates the per-core outputs.
"""

import contextlib
import dataclasses

import numpy as np

import concourse.bass as bass
import concourse.mybir as mybir
import concourse.tile as tile
import concourse.bacc as bacc
from concourse.bass_utils import run_bass_kernel_spmd
from concourse.alu_op_type import AluOpType

F32 = mybir.dt.float32
BF16 = mybir.dt.bfloat16
I16 = mybir.dt.int16

PAD_OFF = 200.0  # dst_off sentinel for padding edges -> one-hot column all-zero


@dataclasses.dataclass
class Cfg:
    N: int = 50000
    E: int = 800000
    IN: int = 256
    HEADS: int = 6
    HID: int = 64
    OUT: int = 32
    NEG: float = 0.2
    NC: int = 8
    NB: int = 49
    BLK: int = 128
    skip_cc: bool = False   # debug: replace AllGather with local copy (wrong results)

    @property
    def D1(self):
        return self.HEADS * self.HID

    @property
    def SHARD(self):
        return self.NB * self.BLK

    @property
    def NPAD(self):
        return self.NC * self.SHARD

    @property
    def HALF(self):
        return self.NPAD // 2

    @property
    def ROW1(self):
        return ((self.D1 + 2 * self.HEADS + 127) // 128) * 128 // 2  # f32/row

    @property
    def ROW1U(self):
        return (self.D1 + 2 * self.HEADS) // 2   # used f32 elems / tab1 row

    @property
    def ROW2(self):
        return 64                                # f32 elems / tab2 row

    @property
    def ROW2U(self):
        return (self.OUT + 2) // 2               # used f32 elems / tab2 row


GRP = 7          # h2e flush / AllGather granularity, in 128-node blocks
SPLITG = 4       # tab2 groups in gather stream A (4*7*128*8 = 28672 rows)


def _wrap_idx(idx_flat):
    """int16 gather index layout: index i at [partition i%16, free i//16],
    replicated down to 128 partitions."""
    n = idx_flat.shape[0]
    assert n % 16 == 0
    w = idx_flat.reshape(-1, 16).T.astype(np.int16)
    return np.tile(w, (8, 1))


def _tab2_geom(cfg: Cfg):
    """tab2 is laid out in GRP-block groups so each group can be
    AllGathered as soon as all cores have produced it: group g holds
    [core0 rows | core1 rows | ...]."""
    ng = (cfg.NB + GRP - 1) // GRP
    gsz = [min(GRP, cfg.NB - g * GRP) * cfg.BLK for g in range(ng)]
    gbase = np.concatenate([[0], np.cumsum([s * cfg.NC for s in gsz])])
    bound = int(gbase[min(SPLITG, ng)])   # stream A/B split row
    return ng, gsz, gbase.astype(np.int64), bound


def _tab2_row(cfg: Cfg, n):
    """Map a global node id to its row in the grouped tab2 layout."""
    ng, gsz, gbase, _ = _tab2_geom(cfg)
    n = np.asarray(n, np.int64)
    c = n // cfg.SHARD
    L = n % cfg.SHARD
    g = np.minimum(L // (GRP * cfg.BLK), ng - 1)
    w = L - g * GRP * cfg.BLK
    sz = np.asarray(gsz, np.int64)[g]
    return gbase[g] + c * sz + w


@dataclasses.dataclass
class EdgePlan:
    chA: list
    chB: list
    nreal: list
    G1: int
    idx: np.ndarray
    dstoff_col: np.ndarray
    dstoff_row: np.ndarray
    totw: int


def _edges(cfg: Cfg, edge_index: np.ndarray):
    src = np.concatenate([np.asarray(edge_index[0], np.int64),
                          np.arange(cfg.N, dtype=np.int64)])
    dst = np.concatenate([np.asarray(edge_index[1], np.int64),
                          np.arange(cfg.N, dtype=np.int64)])
    return src, dst


def build_edge_plan(cfg: Cfg, edge_index: np.ndarray) -> EdgePlan:
    """One plan serves both layers: tab1 and tab2 share the grouped row
    layout, so stream split and gather indices are identical."""
    NC, NB, BLK = cfg.NC, cfg.NB, cfg.BLK
    SHARD = cfg.SHARD
    src, dst = _edges(cfg, edge_index)
    core = dst // SHARD
    blk = (dst % SHARD) // BLK
    off = (dst % BLK).astype(np.float32)
    _, _, _, bound = _tab2_geom(cfg)
    src2 = _tab2_row(cfg, src)
    strm = (src2 >= bound).astype(np.int64)
    lsrc = (src2 - strm * bound).astype(np.int32)

    def own_fn(k, b):
        s = 0 if b < SPLITG * GRP else 1
        return s, int(_tab2_row(cfg, k * SHARD + b * BLK)) - s * bound

    slot = (core * NB + blk) * 2 + strm
    order = np.argsort(slot, kind="stable")
    slot_s, lsrc_s, off_s = slot[order], lsrc[order], off[order]
    counts = np.bincount(slot_s, minlength=NC * NB * 2)
    starts = np.concatenate([[0], np.cumsum(counts)])

    cnt = counts.reshape(NC, NB, 2)
    ch = np.maximum(1, -(-cnt.max(axis=0) // BLK))
    chA, chB = ch[:, 0].tolist(), ch[:, 1].tolist()
    nreal = [a + b for a, b in zip(chA, chB)]
    G1 = int(sum(nreal))
    RW = -(-G1 // 128) * 128
    totw = sum((1 + a + 1 + b) * (BLK // 16) for a, b in zip(chA, chB))

    idx_all = np.zeros((NC, 128, totw), np.int16)
    dcol = np.full((NC, 128, G1), PAD_OFF, np.float32)
    drow = np.full((NC, 128, RW), PAD_OFF, np.float32)

    for k in range(NC):
        wpos = 0
        g = 0
        for b in range(NB):
            for s, nch in ((0, chA[b]), (1, chB[b])):
                seg = np.zeros(((1 + nch) * BLK,), np.int32)
                s_own, own0 = own_fn(k, b)
                if s == s_own:
                    seg[:BLK] = own0 + np.arange(BLK)
                sidx = (k * NB + b) * 2 + s
                st, en = starts[sidx], starts[sidx + 1]
                cntk = en - st
                assert cntk <= nch * BLK
                seg[BLK:BLK + cntk] = lsrc_s[st:en]
                w = _wrap_idx(seg)
                idx_all[k][:, wpos:wpos + w.shape[1]] = w
                wpos += w.shape[1]
                offs = np.full((nch * BLK,), PAD_OFF, np.float32)
                offs[:cntk] = off_s[st:en]
                offs = offs.reshape(nch, BLK)
                for c in range(nch):
                    dcol[k][:, g] = offs[c]
                    drow[k][g % 128, (g // 128) * 128:(g // 128 + 1) * 128] = offs[c]
                    g += 1
        assert g == G1 and wpos == totw
    return EdgePlan(chA, chB, nreal, G1, idx_all, dcol, drow, totw)


def build_nc(cfg: Cfg, plan: EdgePlan):
    c = cfg
    nc = bacc.Bacc("TRN2", target_bir_lowering=False, debug=False,
                   enable_asserts=False, num_devices=c.NC,
                   num_swdge_queues=1)

    H = c.HEADS
    D1, IN, OUT = c.D1, c.IN, c.OUT
    NBK = c.NPAD // 128
    KIN = IN // 128
    K1 = D1 // 128
    NG, GSZ, GBASE, BOUND = _tab2_geom(c)
    NBBND = BOUND // 128          # blocks in the A half

    xt = nc.dram_tensor("xt", [IN, c.NPAD], BF16, kind="ExternalInput")
    w1 = nc.dram_tensor("w1", [IN, D1], BF16, kind="ExternalInput")
    w1t = nc.dram_tensor("w1t", [D1, IN], BF16, kind="ExternalInput")
    attbd1 = nc.dram_tensor("attbd1", [D1, 2 * H], BF16, kind="ExternalInput")
    w2 = nc.dram_tensor("w2", [D1, OUT], BF16, kind="ExternalInput")
    w2t = nc.dram_tensor("w2t", [OUT, D1], BF16, kind="ExternalInput")
    att2 = nc.dram_tensor("att2", [OUT, 2], BF16, kind="ExternalInput")
    b1r = nc.dram_tensor("b1r", [128, D1], F32, kind="ExternalInput")
    b2r = nc.dram_tensor("b2r", [128, OUT], F32, kind="ExternalInput")
    iota_r = nc.dram_tensor("iota_r", [128, 128], BF16, kind="ExternalInput")
    iota_c = nc.dram_tensor("iota_c", [128, 1], F32, kind="ExternalInput")
    ident = nc.dram_tensor("ident", [128, 128], BF16, kind="ExternalInput")
    idx_d = nc.dram_tensor("idx_d", [128, plan.totw], I16, kind="ExternalInput")
    dcol_d = nc.dram_tensor("dcol_d", [128, plan.G1], F32, kind="ExternalInput")

    NBND = max(128, c.NPAD - BOUND)
    tab1a = nc.dram_tensor("tab1a", [BOUND, c.ROW1], F32)
    tab1b = nc.dram_tensor("tab1b", [NBND, c.ROW1], F32)
    h2e_own = nc.dram_tensor("h2e_own", [c.SHARD, c.ROW2U], F32)
    tab2a = nc.dram_tensor("tab2a", [BOUND, c.ROW2], F32)
    tab2b = nc.dram_tensor("tab2b", [NBND, c.ROW2], F32)
    tab2ca = nc.dram_tensor("tab2ca", [BOUND, c.ROW2U], F32,
                            addr_space="Shared")
    tab2cb = nc.dram_tensor("tab2cb", [NBND, c.ROW2U], F32,
                            addr_space="Shared")
    y = nc.dram_tensor("y", [c.SHARD, OUT], F32, kind="ExternalOutput")

    AS0 = D1                 # bf16 col of a_src in a tab1 row
    AD0 = AS0 + H             # bf16 col of a_dst in a tab1 row

    with tile.TileContext(nc, num_cores=c.NC) as tc:
        with contextlib.ExitStack() as ctx:
            consts = ctx.enter_context(tc.tile_pool(name="consts", bufs=1))
            stg = ctx.enter_context(tc.tile_pool(name="stg", bufs=3))
            pmm = ctx.enter_context(tc.tile_pool(name="pmm", bufs=2, space="PSUM"))
            ppa = ctx.enter_context(tc.tile_pool(name="ppa", bufs=3, space="PSUM"))
            paux = ctx.enter_context(tc.tile_pool(name="paux", bufs=3, space="PSUM"))

            def load_const(dram, shape, dtype):
                t = consts.tile(shape, dtype, tag=dram.name)
                nc.sync.dma_start(t[:], dram.ap())
                return t

            iota_row = load_const(iota_r, [128, 128], BF16)
            iota_col = load_const(iota_c, [128, 1], F32)
            ident_b = load_const(ident, [128, 128], BF16)
            b1_sb = load_const(b1r, [128, D1], F32)
            b2_sb = load_const(b2r, [128, OUT], F32)
            dcol_sb = load_const(dcol_d, [128, plan.G1], F32)

            # ---- W1e [128, KIN, D1+2H] and W2e [128, K1, OUT+2] ----
            w1e = consts.tile([128, KIN, D1 + 2 * H], BF16, tag="w1e")
            for ki in range(KIN):
                nc.sync.dma_start(w1e[:, ki, 0:D1],
                                  w1.ap()[ki * 128:(ki + 1) * 128, :])
            w1t_s = consts.tile([128, K1, IN], BF16, tag="w1t_s")
            for kj in range(K1):
                nc.sync.dma_start(w1t_s[:, kj, :],
                                  w1t.ap()[kj * 128:(kj + 1) * 128, :])
            abd_s = consts.tile([128, K1, 2 * H], BF16, tag="abd_s")
            for kj in range(K1):
                nc.sync.dma_start(abd_s[:, kj, :],
                                  attbd1.ap()[kj * 128:(kj + 1) * 128, :])
            for ki in range(KIN):
                ps = paux.tile([128, 2 * H], F32, tag="aux")
                for kj in range(K1):
                    nc.tensor.matmul(ps[:], w1t_s[:, kj, ki * 128:(ki + 1) * 128],
                                     abd_s[:, kj, :], start=(kj == 0),
                                     stop=(kj == K1 - 1))
                nc.scalar.copy(w1e[:, ki, D1:D1 + 2 * H], ps[:])

            w2e = consts.tile([128, K1, OUT + 2], BF16, tag="w2e")
            for kj in range(K1):
                nc.sync.dma_start(w2e[:, kj, 0:OUT],
                                  w2.ap()[kj * 128:(kj + 1) * 128, :])
            w2t_s = consts.tile([128, D1], BF16, tag="w2t_s")
            nc.sync.dma_start(w2t_s[:OUT, :], w2t.ap())
            att2_s = consts.tile([128, 2], BF16, tag="att2_s")
            nc.sync.dma_start(att2_s[:OUT, :], att2.ap())
            for kj in range(K1):
                ps = paux.tile([128, 2], F32, tag="aux")
                nc.tensor.matmul(ps[:], w2t_s[:OUT, kj * 128:(kj + 1) * 128],
                                 att2_s[:OUT, :], start=True, stop=True)
                nc.scalar.copy(w2e[:, kj, OUT:OUT + 2], ps[:])

            if BOUND >= c.NPAD:
                # small configs: stream B tables are dummies that pad
                # gathers still read -- zero them so no NaNs flow through
                zt = consts.tile([128, c.ROW1], F32, tag="zt")
                nc.vector.memset(zt[:], 0.0)
                nc.sync.dma_start(tab1b.ap()[0:128, :], zt[:])
                nc.sync.dma_start(tab2b.ap()[0:128, :], zt[:, 0:c.ROW2])
                nc.sync.dma_start(tab2cb.ap()[0:128, :], zt[:, 0:c.ROW2U])

            # ---- P1: replicated projection -> tab1 ----
            # 8 blocks per xt DMA, 4 blocks per (strided) tab1 store; the
            # PSUM->SBUF copies alternate between ACT and DVE. Row pad bytes
            # are never read by the gather consumers, so they stay garbage.
            assert NBK % 8 == 0
            RU = c.ROW1U
            p1ctx = contextlib.ExitStack()
            projx = p1ctx.enter_context(tc.tile_pool(name="projx", bufs=3))
            p1stg = p1ctx.enter_context(tc.tile_pool(name="p1stg", bufs=2))
            for g8 in range(NBK // 8):
                xts = []
                for ki in range(KIN):
                    xtile = projx.tile([128, 1024], BF16, tag="xt")
                    nc.sync.dma_start(
                        xtile[:], xt.ap()[ki * 128:(ki + 1) * 128,
                                          g8 * 1024:(g8 + 1) * 1024])
                    xts.append(xtile)
                st8 = p1stg.tile([128, 8, c.ROW1], F32, tag="stage1")
                for j in range(8):
                    ps = pmm.tile([128, D1 + 2 * H], F32, tag="mm")
                    for ki in range(KIN):
                        nc.tensor.matmul(
                            ps[:], xts[ki][:, j * 128:(j + 1) * 128],
                            w1e[:, ki, :], start=(ki == 0),
                            stop=(ki == KIN - 1))
                    if j % 2 == 0:
                        nc.scalar.copy(st8[:, j, 0:RU].bitcast(BF16), ps[:])
                    else:
                        nc.vector.tensor_copy(st8[:, j, 0:RU].bitcast(BF16),
                                              ps[:])
                nb0 = g8 * 8
                r0 = nb0 * 128
                if r0 < BOUND:
                    tdst = tab1a.ap()[r0:r0 + 1024, 0:RU]
                else:
                    tdst = tab1b.ap()[r0 - BOUND:r0 - BOUND + 1024, 0:RU]
                nc.sync.dma_start(tdst.rearrange("(b p) e -> p b e", b=8),
                                  st8[:, :, 0:RU])
            p1ctx.close()
            # edge-phase pools open after P1's close so they reuse its SBUF
            gp = ctx.enter_context(tc.tile_pool(name="gath", bufs=5))
            up = ctx.enter_context(tc.tile_pool(name="upool", bufs=4))
            utp = ctx.enter_context(tc.tile_pool(name="utp", bufs=6))
            hwp = ctx.enter_context(tc.tile_pool(name="hwp", bufs=3))
            wk = ctx.enter_context(tc.tile_pool(name="wk", bufs=4))
            idxp = ctx.enter_context(tc.tile_pool(name="idxp", bufs=6))

            # no barrier: Tile's DRAM RAW tracking orders the tab1 gathers
            # after the last tab1 store, while idx loads / one-hot builds /
            # transposes for early blocks overlap the P1 tail

            # ---- shared edge phase ----
            self_q = [0]

            def edge_phase(pl, idxd, dcol_t, tabv_a, tabv_b, row_elems,
                           nh, chans, as_col, ad_col, out_cb, tag,
                           own_s_fn=None, block_cb=None):
                wseg = 0
                g = 0
                for b in range(c.NB):
                    nA, nB_ = pl.chA[b], pl.chB[b]
                    nr = nA + nB_
                    niA, niB = (1 + nA) * 8, (1 + nB_) * 8
                    it = idxp.tile([128, niA + niB], I16, tag="idx")
                    nc.sync.dma_start(it[:],
                                      idxd.ap()[:, wseg:wseg + niA + niB])
                    wseg += niA + niB
                    s_own = own_s_fn(b)
                    gts = []
                    g0 = gp.tile([128, 1, row_elems], F32, tag="g0")
                    for s, nch, io in ((0, nA, 0), (1, nB_, niA)):
                        # row 0 (own block rows for the a_dst read) goes to
                        # its own small tile so a_dst doesn't wait the full
                        # gather; ring holds 1024 descs -> pieces of <= 7
                        # chunks (896)
                        if s == s_own:
                            gather_box[0] = nc.gpsimd.dma_gather(
                                g0[:], tabv_a if s == 0 else tabv_b,
                                it[:, io:io + 8], 128, 128, row_elems,
                                queue_num=0)
                        gt = gp.tile([128, nch, row_elems], F32, tag=f"g{s}")
                        po = 0
                        while po < nch:
                            pc = min(7, nch - po)
                            gather_box[0] = nc.gpsimd.dma_gather(
                                gt[:, po:po + pc, :],
                                tabv_a if s == 0 else tabv_b,
                                it[:, io + (po + 1) * 8:io + (po + pc + 1) * 8],
                                pc * 128, pc * 128, row_elems,
                                queue_num=0)
                            po += pc
                        gts.append(gt)
                    gA, gB = gts
                    if block_cb is not None:
                        block_cb(b)

                    adst_b = wk.tile([128, nh], BF16, tag="adstb")
                    nc.vector.tensor_copy(
                        adst_b[:],
                        g0[:, 0:1, :].bitcast(BF16)[:, 0, ad_col:ad_col + nh])

                    # U one-hots for all chunks; UT via PE transpose, 8 chunks
                    # batched per PSUM tile (one bank) + one ACT copy to SBUF.
                    uall = up.tile([128, nr * 128], BF16, tag="ua")
                    uts = []
                    for r8 in range(0, nr, 8):
                        cnt = min(8, nr - r8)
                        pst8 = paux.tile([128, 8, 128], BF16, tag="aux")
                        for j in range(cnt):
                            gg = g + r8 + j
                            us = uall[:, (r8 + j) * 128:(r8 + j + 1) * 128]
                            nc.vector.tensor_scalar(
                                us, iota_row[:], dcol_t[:, gg:gg + 1], None,
                                op0=AluOpType.is_equal)
                            nc.tensor.transpose(pst8[:, j, :], us, ident_b[:])
                        ut8 = utp.tile([128, 8, 128], BF16, tag="ut")
                        if tag == "1" and b < 8:
                            # ACT is saturated by the P1 copies early on
                            nc.vector.tensor_copy(ut8[:, 0:cnt, :],
                                                  pst8[:, 0:cnt, :])
                        else:
                            nc.scalar.copy(ut8[:, 0:cnt, :], pst8[:, 0:cnt, :])
                        uts.append(ut8)

                    # per stream: paE (edge a_dst via tiny PE matmuls),
                    # esum = a_src + a_dst_edge, fused leaky-relu, exp,
                    # hw = [ex * h | ex], then the aggregation matmuls
                    pnum = pmm.tile([128, chans + nh], F32, tag="mm")
                    for s, nch, base in ((0, nA, 0), (1, nB_, nA)):
                        gt = gA if s == 0 else gB
                        paE = ppa.tile([128, nh * nch], F32, name="paE",
                                       tag="pa")
                        for rl in range(nch):
                            r = base + rl
                            nc.tensor.matmul(paE[:, rl * nh:(rl + 1) * nh],
                                             uts[r // 8][:, r % 8, :],
                                             adst_b[:], start=True, stop=True)
                        gbv = gt[:, :, :].bitcast(BF16)
                        asrc = gbv[:, :, as_col:as_col + nh]
                        lk = wk.tile([128, nh * nch], F32, name="lk",
                                     tag="lk")
                        pv = paE[:].rearrange("p (ch h) -> p ch h", h=nh)
                        ev = lk[:].rearrange("p (ch h) -> p ch h", h=nh)
                        nc.vector.tensor_tensor(ev, asrc, pv, op=AluOpType.add)
                        # leaky relu in-place: max(neg*x, x)
                        nc.vector.scalar_tensor_tensor(
                            lk[:], lk[:], c.NEG, lk[:],
                            op0=AluOpType.mult, op1=AluOpType.max)
                        # exp straight into the hw tile's trailing ex
                        # columns (doubles as the denominator input)
                        hw = hwp.tile([128, nch, chans + nh], BF16,
                                      tag=f"hw{s}")
                        exd = hw[:, :, chans:chans + nh]
                        nc.scalar.activation(
                            exd, lk[:].rearrange("p (ch h) -> p ch h", h=nh),
                            mybir.ActivationFunctionType.Exp)
                        hv = gbv[:, :, 0:chans].rearrange(
                            "p ch (h w) -> p ch h w", h=nh)
                        exb = exd.broadcast_to([128, nch, nh, chans // nh])
                        hwv = hw[:, :, 0:chans].rearrange(
                            "p ch (h w) -> p ch h w", h=nh)
                        nc.vector.tensor_tensor(hwv, hv, exb,
                                                op=AluOpType.mult)
                        for rl in range(nch):
                            r = base + rl
                            U = uall[:, r * 128:(r + 1) * 128]
                            nc.tensor.matmul(pnum[:], U, hw[:, rl, :],
                                             start=(r == 0),
                                             stop=(r == nr - 1))
                    out_cb(b, pnum)
                    g += nr

            # ---- P2: layer-1 edges (+ fused layer-2 projection) ----
            tabA1 = tab1a.ap()
            tabB1 = tab1b.ap()
            own_s = lambda b: 0 if b < SPLITG * GRP else 1
            st2_box = [None]
            st_box = [None] * NG
            gather_box = [None]

            def l1_block_cb(b):
                # AllGather of group gi fires three blocks after its flush
                # so the next blocks' gathers are already in flight when the
                # CC occupies the Pool engine
                if b % GRP == 2 and b >= GRP:
                    emit_cc(b // GRP - 1)

            def l1_out(b, pnum):
                den = wk.tile([128, H], F32, tag="den1")
                nc.vector.tensor_scalar(den[:], pnum[:, D1:D1 + H], 1e-30, None,
                                        op0=AluOpType.max)
                rec = wk.tile([128, H], F32, tag="rec1")
                nc.vector.reciprocal(rec[:], den[:])
                tmp = wk.tile([128, D1], F32, tag="tmp1")
                nv = pnum[:, 0:D1].rearrange("p (h ch) -> p h ch", h=H)
                rb = rec[:].broadcast_to([128, H, c.HID])
                tv = tmp[:].rearrange("p (h ch) -> p h ch", h=H)
                nc.vector.tensor_tensor(tv, nv, rb, op=AluOpType.mult)
                nc.vector.tensor_tensor(tmp[:], tmp[:], b1_sb[:],
                                        op=AluOpType.add)
                h1s = wk.tile([128, D1], BF16, tag="h1s")
                nc.scalar.activation(h1s[:], tmp[:],
                                     mybir.ActivationFunctionType.Relu)
                hps = paux.tile([128, K1, 128], BF16, tag="aux")
                for j in range(K1):
                    nc.tensor.transpose(hps[:, j, :],
                                        h1s[:, j * 128:(j + 1) * 128],
                                        ident_b[:])
                h1b = wk.tile([128, K1, 128], BF16, tag="h1b")
                nc.scalar.copy(h1b[:], hps[:])
                # fused P3: h2e row block = [h1 @ W2 | a2] for this block
                ps2 = paux.tile([128, OUT + 2], F32, tag="aux")
                for kj in range(K1):
                    nc.tensor.matmul(ps2[:], h1b[:, kj, :],
                                     w2e[:, kj, :], start=(kj == 0),
                                     stop=(kj == K1 - 1))
                if b % GRP == 0:
                    st2_box[0] = stg.tile([128, GRP, c.ROW2U], F32,
                                          name="st2g", tag="stage2")
                st2 = st2_box[0]
                j = b % GRP
                nc.vector.tensor_copy(st2[:, j, :].bitcast(BF16), ps2[:])
                if b % GRP == GRP - 1 or b == c.NB - 1:
                    # flush this group of h2e rows; the AllGather of group
                    # gi is issued one block later (see emit_cc) so the next
                    # block's gathers are already in flight when the CC
                    # occupies the Pool engine
                    nbat = j + 1
                    b0 = b - j
                    gi = b0 // GRP
                    dst = h2e_own.ap()[b0 * 128:(b0 + nbat) * 128, :] \
                        .rearrange("(b p) e -> p b e", b=nbat)
                    st_box[gi] = nc.sync.dma_start(dst, st2[:, 0:nbat, :])

            def emit_cc(gi):
                RU2 = c.ROW2U
                b0 = gi * GRP
                gb = int(GBASE[gi])
                t2, t2c = (tab2a, tab2ca) if gb < BOUND else (tab2b, tab2cb)
                if gb >= BOUND:
                    gb -= BOUND
                nrows = c.NC * GSZ[gi]
                ins_ap = h2e_own.ap()[b0 * 128:b0 * 128 + GSZ[gi], :]
                out_ap = t2c.ap()[gb:gb + nrows, :]
                if c.skip_cc:
                    for q in range(c.NC):
                        cp = nc.sync.dma_start(
                            t2c.ap()[gb + q * GSZ[gi]:gb + (q + 1) * GSZ[gi],
                                     :], ins_ap)
                        tile.add_dep_helper(cp.ins, st_box[gi].ins,
                                            reason="skip_cc after store")
                else:
                    cc = nc.gpsimd.collective_compute(
                        "AllGather", AluOpType.bypass,
                        replica_groups=[list(range(c.NC))],
                        ins=[ins_ap], outs=[out_ap])
                    tile.add_dep_helper(cc.ins, st_box[gi].ins,
                                        reason="AllGather after store")
                    if gather_box[0] is not None:
                        tile.add_dep_helper(cc.ins, gather_box[0].ins,
                                            sync=False,
                                            reason="CC after next gathers")
                # expand compact rows into the 256B-aligned gather table
                nc.sync.dma_start(t2.ap()[gb:gb + nrows, 0:RU2],
                                  t2c.ap()[gb:gb + nrows, :])

            edge_phase(plan, idx_d, dcol_sb, tabA1, tabB1, c.ROW1, H, D1,
                       AS0, AD0, l1_out, "1", own_s_fn=own_s,
                       block_cb=l1_block_cb)
            emit_cc(NG - 1)

            # ---- P4: layer-2 edges (gathers wait CC writers via deps) ----
            tabA2 = tab2a.ap()
            tabB2 = tab2b.ap()
            AS2 = OUT       # bf16 col of a_src2 in a tab2 row
            sty_box = [None]

            def l2_out(b, pnum):
                den = wk.tile([128, 1], F32, tag="den2")
                nc.vector.tensor_scalar(den[:], pnum[:, OUT:OUT + 1], 1e-30,
                                        None, op0=AluOpType.max)
                rec = wk.tile([128, 1], F32, tag="rec2")
                nc.vector.reciprocal(rec[:], den[:])
                if b % 4 == 0:
                    sty_box[0] = stg.tile([128, 4, OUT], F32,
                                          name="styg", tag="stagey")
                sty = sty_box[0]
                j = b % 4
                nc.vector.tensor_scalar(sty[:, j, :], pnum[:, 0:OUT],
                                        rec[:, 0:1], None, op0=AluOpType.mult)
                nc.vector.tensor_tensor(sty[:, j, :], sty[:, j, :], b2_sb[:],
                                        op=AluOpType.add)
                if b % 4 == 3 or b == c.NB - 1:
                    nbat = j + 1
                    b0 = b - j
                    dst = y.ap()[b0 * 128:(b0 + nbat) * 128, :] \
                        .rearrange("(b p) e -> p b e", b=nbat)
                    nc.sync.dma_start(dst, sty[:, 0:nbat, :])

            edge_phase(plan, idx_d, dcol_sb, tabA2, tabB2, c.ROW2, 1,
                       OUT, AS2, AS2 + 1, l2_out, "2", own_s_fn=own_s)

    nc.compile()
    return nc


def host_inputs(cfg: Cfg, plan: EdgePlan, x, W1, att_src1,
                att_dst1, b1, W2, att_src2, att_dst2, b2):
    c = cfg
    H = c.HEADS

    def bf(a):
        import ml_dtypes
        return np.asarray(a, np.float32).astype(ml_dtypes.bfloat16)

    xt = np.zeros((c.IN, c.NPAD), np.float32)
    perm = _tab2_row(c, np.arange(c.N))
    xt[:, perm] = np.asarray(x, np.float32).T
    attbd1 = np.zeros((c.D1, 2 * H), np.float32)
    a_s1 = np.asarray(att_src1, np.float32).reshape(H, c.HID)
    a_d1 = np.asarray(att_dst1, np.float32).reshape(H, c.HID)
    for h in range(H):
        attbd1[h * c.HID:(h + 1) * c.HID, h] = a_s1[h]
        attbd1[h * c.HID:(h + 1) * c.HID, H + h] = a_d1[h]
    att2 = np.stack([np.asarray(att_src2, np.float32).reshape(c.OUT),
                     np.asarray(att_dst2, np.float32).reshape(c.OUT)], axis=1)

    base = {
        "xt": bf(xt),
        "w1": bf(W1),
        "w1t": bf(np.ascontiguousarray(np.asarray(W1, np.float32).T)),
        "attbd1": bf(attbd1),
        "w2": bf(W2),
        "w2t": bf(np.ascontiguousarray(np.asarray(W2, np.float32).T)),
        "att2": bf(att2),
        "b1r": np.tile(np.asarray(b1, np.float32).reshape(1, c.D1), (128, 1)),
        "b2r": np.tile(np.asarray(b2, np.float32).reshape(1, c.OUT), (128, 1)),
        "iota_r": bf(np.tile(np.arange(128, dtype=np.float32)[None, :],
                             (128, 1))),
        "iota_c": np.arange(128, dtype=np.float32)[:, None],
        "ident": bf(np.eye(128, dtype=np.float32)),
    }
    in_maps = []
    for k in range(c.NC):
        m = dict(base)
        m["idx_d"] = plan.idx[k]
        m["dcol_d"] = plan.dstoff_col[k]
        in_maps.append(m)
    return in_maps


_CACHE = {}
LAST_RES = None


def kernel(x, edge_index, W1, att_src1, att_dst1, b1, W2, att_src2, att_dst2,
           b2, _cfg=None, _runner=None, _trace=False):
    cfg = _cfg or Cfg()
    ei = np.asarray(edge_index)
    plan = build_edge_plan(cfg, ei)
    key = (cfg.N, cfg.E, cfg.skip_cc, tuple(plan.chA), tuple(plan.chB))
    if key not in _CACHE:
        _CACHE[key] = build_nc(cfg, plan)
    nc = _CACHE[key]
    in_maps = host_inputs(cfg, plan, x, W1, att_src1, att_dst1, b1, W2,
                          att_src2, att_dst2, b2)
    global LAST_RES
    if _runner is not None:
        results = _runner(nc, in_maps)
    else:
        try:
            res = run_bass_kernel_spmd(nc, in_maps,
                                       core_ids=list(range(cfg.NC)),
                                       trace=_trace)
        except ModuleNotFoundError:
            if not _trace:
                raise
            # NTFF profiling hook unavailable in this environment
            res = run_bass_kernel_spmd(nc, in_maps,
                                       core_ids=list(range(cfg.NC)))
        LAST_RES = res
        results = res.results
    out = np.concatenate([results[k]["y"] for k in range(cfg.NC)], axis=0)
    return np.ascontiguousarray(out[:cfg.N]).astype(np.float32)
